# revision 14
# baseline (speedup 1.0000x reference)
"""Multi-head attention (RoPE, 16 heads, D=1024, B=2, N=2048) on 8 trn2 cores.

Sharding: core c handles batch b = c//4 and heads [4*(c%4), 4*(c%4)+4).
Each core computes its 4 heads' attention plus its partial out-projection
(columns of Wo for its heads); host sums the 4 partials per batch.

Layout strategy (per core):
  - Q^T, K^T computed directly in (head_dim, tokens) layout; head_dim rows are
    permuted (evens then odds) via host-permuted Wq/Wk rows so RoPE becomes a
    rotate-half over partitions 0:32/32:64 of each 64-row head block.
  - scores^T tiles (k_tokens x q) via row-packed K=64 matmuls (head pairs at
    partitions 0:64 / 64:128 of shared Q^T/K^T tiles).
  - exp on ScalarE straight out of PSUM with fused 1/sqrt(hd) scale.
  - numerator matmul uses V augmented with a ones column (M=65): row 64 of the
    PSUM accumulator is the softmax denominator.
  - normalize: DVE reciprocal of row 64, gpsimd partition-broadcast, DVE mult.
  - out-projection: lhsT = normalized attention output (already transposed),
    rhs = Wo columns for this core's heads, accumulated over 2 head-pair chunks.
"""

import numpy as np

import concourse.bass as bass
import concourse.mybir as mybir
import concourse.tile as tile
from concourse import bacc
from concourse.bass_utils import run_bass_kernel_spmd

F32 = mybir.dt.float32
F32R = mybir.dt.float32r
AF = mybir.ActivationFunctionType
OP = mybir.AluOpType

B, N, D = 2, 2048, 1024
H, HD = 16, 64
HPC = 4  # heads per core
N_CORES = 8
SCALE = HD ** -0.5

N_TOKTILES = N // 128      # 16
N_KTILES = N // 128        # 16
N_QTILES = N // 512        # 4
N_DTILES = D // 128        # 8
KG = 2                     # ktiles per exp group
NEG = -1.0e30


def _build_program(mask_all_ones: bool):
    nc = bacc.Bacc("TRN2", target_bir_lowering=False, debug=False)

    xT = nc.dram_tensor("xT", [D, N], F32R, kind="ExternalInput")
    wqT = nc.dram_tensor("wqT", [D, HPC * HD], F32R, kind="ExternalInput")
    wkT = nc.dram_tensor("wkT", [D, HPC * HD], F32R, kind="ExternalInput")
    wvT = nc.dram_tensor("wvT", [D, HPC * HD], F32R, kind="ExternalInput")
    woT = nc.dram_tensor("woT", [HPC * HD, D], F32R, kind="ExternalInput")
    cosT = nc.dram_tensor("cosT", [128, N], F32, kind="ExternalInput")
    sinT = nc.dram_tensor("sinT", [128, N], F32, kind="ExternalInput")
    mmul = nc.dram_tensor("mmul", [128, N_KTILES], F32, kind="ExternalInput")
    zpad = nc.dram_tensor("zpad", [64, N], F32R, kind="ExternalInput")
    y = nc.dram_tensor("y", [N, D], F32, kind="ExternalOutput")

    with tile.TileContext(nc) as tc:
        # ---------------- persistent pools ----------------
        with (
            tc.tile_pool(name="qk", bufs=2 * HPC) as qk_pool,
            tc.tile_pool(name="vaug", bufs=N_TOKTILES) as v_pool,
            tc.tile_pool(name="outT", bufs=2) as outT_pool,
            tc.tile_pool(name="wo", bufs=2) as wo_pool,
            tc.tile_pool(name="mm", bufs=1) as mm_pool,
            tc.tile_pool(name="tab", bufs=2) as tab_pool,
        ):
            # QTp[h], KTp[h]: (128, N) f32r; rows 0:64 = head h, 64:128 = zeros
            # (zero-padded so score matmuls have K=128 and count as HAM-busy)
            QTp = [qk_pool.tile([128, N], F32R, tag="qk", name=f"QTp{_}") for _ in range(HPC)]
            KTp = [qk_pool.tile([128, N], F32R, tag="qk", name=f"KTp{_}") for _ in range(HPC)]
            vaug = [
                v_pool.tile([128, HPC * (HD + 1)], F32R, tag="vaug", name=f"vaug{_}")
                for _ in range(N_TOKTILES)
            ]
            outT = [outT_pool.tile([128, N], F32R, tag="outT", name=f"outT{_}") for _ in range(2)]
            woT_sb = [wo_pool.tile([128, D], F32R, tag="wo", name=f"woTsb{_}") for _ in range(2)]
            mmul_sb = mm_pool.tile([128, N_KTILES], F32)
            cos_sb = tab_pool.tile([128, N], F32, tag="tab")
            sin_sb = tab_pool.tile([128, N], F32, tag="tab")

            # zero-pad rows via the scalar HWDGE queue (idle at start); RoPE
            # tables on gpsimd
            for h in range(HPC):
                nc.scalar.dma_start(QTp[h][64:128, :], zpad.ap()[:])
                nc.scalar.dma_start(KTp[h][64:128, :], zpad.ap()[:])
            nc.gpsimd.dma_start(cos_sb[:], cosT.ap()[:])
            nc.gpsimd.dma_start(sin_sb[:], sinT.ap()[:])
            if not mask_all_ones:
                nc.gpsimd.dma_start(mmul_sb[:], mmul.ap()[:])

            # ---------------- phase A: projections + RoPE (per quarter) ------
            with (
                tc.tile_pool(name="raw", bufs=6) as raw_pool,
                tc.tile_pool(name="rot", bufs=4) as rot_pool,
                tc.tile_pool(name="xt", bufs=2 * N_DTILES) as xt_pool,
                tc.tile_pool(name="w", bufs=N_DTILES) as w_pool,
                tc.tile_pool(name="on", bufs=1) as on_pool,
                tc.tile_pool(name="ppsum", bufs=4, space="PSUM") as ppsum,
                tc.tile_pool(name="vpsum", bufs=2, space="PSUM") as vpsum,
            ):
                wq_sb = [w_pool.tile([128, HPC * HD], F32R, tag="wq", name=f"wq{_}") for _ in range(N_DTILES)]
                wk_sb = [w_pool.tile([128, HPC * HD], F32R, tag="wk", name=f"wk{_}") for _ in range(N_DTILES)]
                wv_sb = [w_pool.tile([128, HPC * HD], F32R, tag="wv", name=f"wv{_}") for _ in range(N_DTILES)]
                for d in range(N_DTILES):
                    nc.sync.dma_start(wq_sb[d][:], wqT.ap()[d * 128:(d + 1) * 128, :])
                NQ = N // 4  # 512-token quarters for xT residency
                xt0 = [xt_pool.tile([128, NQ], F32R, tag="xt", name=f"xt0_{_}") for _ in range(N_DTILES)]
                for d in range(N_DTILES):
                    nc.sync.dma_start(xt0[d][:], xT.ap()[d * 128:(d + 1) * 128, 0:NQ])
                for d in range(N_DTILES):
                    nc.sync.dma_start(wk_sb[d][:], wkT.ap()[d * 128:(d + 1) * 128, :])
                for d in range(N_DTILES):
                    nc.sync.dma_start(wv_sb[d][:], wvT.ap()[d * 128:(d + 1) * 128, :])

                ones_sc = on_pool.tile([128, HPC], F32, name="ones_sc")
                nc.vector.memset(ones_sc[:], 1.0)

                for quarter in range(4):
                    qsl = slice(quarter * NQ, (quarter + 1) * NQ)
                    if quarter == 0:
                        xt = xt0
                    else:
                        xt = [xt_pool.tile([128, NQ], F32R, tag="xt", name=f"xt{_}") for _ in range(N_DTILES)]
                        for d in range(N_DTILES):
                            nc.sync.dma_start(xt[d][:], xT.ap()[d * 128:(d + 1) * 128, qsl])
                    # Q^T / K^T projections, then RoPE per (pair, quarter)
                    for ti, w_sb in ((0, wq_sb), (2, wk_sb)):
                        for ch in range(2):
                            ps = ppsum.tile([128, 512], F32, tag="ppsum", name="ps_proj")
                            for d in range(N_DTILES):
                                nc.tensor.matmul(
                                    ps[:],
                                    w_sb[d][:, ch * 128:(ch + 1) * 128],
                                    xt[d][:],
                                    start=(d == 0),
                                    stop=(d == N_DTILES - 1),
                                )
                            rq = raw_pool.tile([128, NQ], F32R, tag="raw", name="rq")
                            nc.scalar.copy(rq[:], ps[:])
                            rot = rot_pool.tile([128, NQ], F32R, tag="rot", name="rot_t")
                            for blk in range(2):
                                b0 = blk * 64
                                nc.scalar.dma_start(rot[b0:b0 + 32, :], rq[b0 + 32:b0 + 64, :])
                                nc.scalar.dma_start(rot[b0 + 32:b0 + 64, :], rq[b0:b0 + 32, :])
                            nc.vector.tensor_tensor(rq[:], rq[:], cos_sb[:, qsl], OP.mult)
                            nc.vector.tensor_tensor(rot[:], rot[:], sin_sb[:, qsl], OP.mult)
                            nc.vector.tensor_tensor(rq[:], rq[:], rot[:], OP.add)
                            dsts = (QTp, KTp)[ti // 2]
                            h0 = ch * 2
                            nc.scalar.dma_start(dsts[h0][0:64, qsl], rq[0:64, :])
                            nc.scalar.dma_start(dsts[h0 + 1][0:64, qsl], rq[64:128, :])
                    # V: out rows = tokens, cols = 4 heads x 64, strided into vaug
                    for tt in range(4):
                        g = quarter * 4 + tt
                        psv = vpsum.tile([128, HPC * HD], F32, tag="vpsum", name="ps_v")
                        for d in range(N_DTILES):
                            nc.tensor.matmul(
                                psv[:],
                                xt[d][:, tt * 128:(tt + 1) * 128],
                                wv_sb[d][:],
                                start=(d == 0),
                                stop=(d == N_DTILES - 1),
                            )
                        for h in range(HPC):
                            nc.scalar.copy(
                                vaug[g][:, h * 65:h * 65 + 64],
                                psv[:, h * 64:(h + 1) * 64],
                            )
                        nc.vector.tensor_copy(vaug[g][:, 64::65], ones_sc[:])

            # ---------------- phase B: attention + inline out-projection -----
            with (
                tc.tile_pool(name="exp", bufs=4) as exp_pool,
                tc.tile_pool(name="div", bufs=4) as div_pool,
                tc.tile_pool(name="yout", bufs=3) as y_pool,
                tc.tile_pool(name="psumS", bufs=2, space="PSUM") as psumS,
                tc.tile_pool(name="psumN", bufs=2, space="PSUM") as psumN,
                tc.tile_pool(name="psumW", bufs=1, space="PSUM") as psumW,
            ):
                for ch in range(2):
                    nc.scalar.dma_start(woT_sb[ch][:], woT.ap()[ch * 128:(ch + 1) * 128, :])

                def emit_wo(q):
                    # out-projection for qtile q (deferred; runs amid later work)
                    for tt in range(4):
                        t0 = q * 512 + tt * 128
                        pw = psumW.tile([128, D], F32, tag="pw", name="pw_t")
                        for nh in range(2):
                            ns = slice(nh * 512, (nh + 1) * 512)
                            for ch in range(2):
                                nc.tensor.matmul(
                                    pw[:, ns],
                                    outT[ch][:, t0:t0 + 128],
                                    woT_sb[ch][:, ns],
                                    start=(ch == 0), stop=(ch == 1),
                                )
                        yt = y_pool.tile([128, D], F32, tag="y", name="y_t")
                        nc.vector.tensor_copy(yt[:], pw[:])
                        nc.gpsimd.dma_start(y.ap()[t0:t0 + 128, :], yt[:])

                for q in range(N_QTILES):
                    qs = slice(q * 512, (q + 1) * 512)
                    for hp in range(2):
                        if hp == 1 and q > 0:
                            emit_wo(q - 1)
                        pnE = psumN.tile([65, 512], F32, tag="pn", name="pnE")
                        pnO = psumN.tile([65, 512], F32, tag="pn", name="pnO")
                        hE, hO = 2 * hp, 2 * hp + 1
                        for kg in range(N_KTILES // KG):
                            psE = psumS.tile([128, 512 * KG], F32, tag="ps", name="psE")
                            psO = psumS.tile([128, 512 * KG], F32, tag="ps", name="psO")
                            for j in range(KG):
                                kt = kg * KG + j
                                ks = slice(kt * 128, (kt + 1) * 128)
                                js = slice(j * 512, (j + 1) * 512)
                                # scores^T, K padded to 128 (rows 64:128 zero)
                                nc.tensor.matmul(
                                    psE[:, js], KTp[hE][:, ks], QTp[hE][:, qs],
                                    start=True, stop=True,
                                )
                                nc.tensor.matmul(
                                    psO[:, js], KTp[hO][:, ks], QTp[hO][:, qs],
                                    start=True, stop=True,
                                )
                            eE = exp_pool.tile([128, 512 * KG], F32R, tag="exp", name="eE")
                            eO = exp_pool.tile([128, 512 * KG], F32R, tag="exp", name="eO")
                            nc.scalar.activation(eE[:], psE[:], AF.Exp, scale=SCALE)
                            nc.scalar.activation(eO[:], psO[:], AF.Exp, scale=SCALE)
                            if not mask_all_ones:
                                for j in range(KG):
                                    kt = kg * KG + j
                                    js = slice(j * 512, (j + 1) * 512)
                                    nc.vector.tensor_scalar_mul(
                                        eE[:, js], eE[:, js], mmul_sb[:, kt:kt + 1]
                                    )
                                    nc.vector.tensor_scalar_mul(
                                        eO[:, js], eO[:, js], mmul_sb[:, kt:kt + 1]
                                    )
                            for j in range(KG):
                                kt = kg * KG + j
                                js = slice(j * 512, (j + 1) * 512)
                                nc.tensor.matmul(
                                    pnE[:], vaug[kt][:, hE * 65:(hE + 1) * 65],
                                    eE[:, js],
                                    start=(kt == 0), stop=(kt == N_KTILES - 1),
                                )
                                nc.tensor.matmul(
                                    pnO[:], vaug[kt][:, hO * 65:(hO + 1) * 65],
                                    eO[:, js],
                                    start=(kt == 0), stop=(kt == N_KTILES - 1),
                                )
                        # stage accumulators out of PSUM first (releases the
                        # pn banks for the next iteration), then normalize
                        stgs = []
                        for pn in (pnE, pnO):
                            stg = div_pool.tile([65, 512], F32, tag="stg", bufs=4, name="stg_t")
                            nc.vector.tensor_copy(stg[:], pn[:])
                            stgs.append(stg)
                        recs = []
                        for stg in stgs:
                            rec = div_pool.tile([1, 512], F32, tag="rec", bufs=2, name="rec_t")
                            nc.vector.reciprocal(rec[:], stg[64:65, :])
                            recs.append(rec)
                        for i, (stg, rec) in enumerate(zip(stgs, recs)):
                            rbc = div_pool.tile([64, 512], F32, tag="rbc", bufs=2, name="rbc_t")
                            nc.gpsimd.partition_broadcast(rbc[:], rec[:])
                            if i == 0:
                                nc.vector.tensor_tensor(
                                    outT[hp][0:64, qs], stg[0:64, :], rbc[:], OP.mult
                                )
                            else:
                                tmp = div_pool.tile([64, 512], F32R, tag="tmp", bufs=2, name="tmp_t")
                                nc.vector.tensor_tensor(
                                    tmp[:], stg[0:64, :], rbc[:], OP.mult
                                )
                                nc.sync.dma_start(outT[hp][64:128, qs], tmp[:])
                emit_wo(N_QTILES - 1)

    nc.compile()
    return nc


_CACHE = {}


def _get_program(mask_all_ones: bool):
    if mask_all_ones not in _CACHE:
        _CACHE[mask_all_ones] = _build_program(mask_all_ones)
    return _CACHE[mask_all_ones]


def _host_inputs(x, mask, Wq, Wk, Wv, Wo):
    """Build the 8 per-core input maps."""
    x = np.asarray(x, np.float32)
    mask = np.asarray(mask)
    Wq, Wk, Wv, Wo = (np.asarray(w, np.float32) for w in (Wq, Wk, Wv, Wo))

    # RoPE tables in rotate-half permuted space, repeated per 64-row block
    inv_freq = 1.0 / (10000.0 ** (np.arange(0, HD, 2, dtype=np.float32) / HD))
    ang = np.outer(np.arange(N, dtype=np.float32), inv_freq)  # (N, 32)
    cos = np.cos(ang).T.astype(np.float32)  # (32, N)
    sin = np.sin(ang).T.astype(np.float32)
    cosT = np.concatenate([cos, cos, cos, cos], 0)  # (128, N)
    sinT = np.concatenate([-sin, sin, -sin, sin], 0)

    perm = np.concatenate([np.arange(0, HD, 2), np.arange(1, HD, 2)])  # evens|odds

    xTs = [np.ascontiguousarray(x[b].T) for b in range(B)]
    in_maps = []
    for c in range(N_CORES):
        b, g = divmod(c, HPC)
        rows = []
        for h in range(HPC):
            h_abs = g * HPC + h
            rows.append(h_abs * HD + perm)
        rows = np.concatenate(rows)  # 256 permuted row indices
        vrows = np.arange(g * HPC * HD, (g + 1) * HPC * HD)  # unpermuted
        mb = mask[b].astype(np.float32).reshape(N_KTILES, 128).T.copy()  # (128,16)
        in_maps.append({
            "xT": xTs[b],
            "wqT": np.ascontiguousarray(Wq[rows].T),
            "wkT": np.ascontiguousarray(Wk[rows].T),
            "wvT": np.ascontiguousarray(Wv[vrows].T),
            "woT": np.ascontiguousarray(Wo[:, vrows].T),
            "cosT": cosT,
            "sinT": sinT,
            "mmul": np.ascontiguousarray(mb),
            "zpad": np.zeros((64, N), np.float32),
        })
    return in_maps


def kernel(x, mask, Wq, Wk, Wv, Wo, _want_profile=False):
    mask_all_ones = bool(np.asarray(mask).all())
    nc = _get_program(mask_all_ones)
    in_maps = _host_inputs(x, mask, Wq, Wk, Wv, Wo)
    kw = {}
    if _want_profile:
        import os
        import shutil
        shutil.rmtree("/root/problem/prof", ignore_errors=True)
        os.makedirs("/root/problem/prof", exist_ok=True)
        kw["tmpdir"] = "/root/problem/prof"
    res = run_bass_kernel_spmd(
        nc, in_maps, list(range(N_CORES)), trace=_want_profile, **kw
    )
    out = np.zeros((B, N, D), np.float32)
    for c in range(N_CORES):
        out[c // HPC] += res.results[c]["y"]
    if _want_profile:
        return out, res
    return out


# revision 15
# speedup vs baseline: 1.0449x; 1.0449x over previous
"""Multi-head attention (RoPE, 16 heads, D=1024, B=2, N=2048) on 8 trn2 cores.

Sharding: core c handles batch b = c//4 and heads [4*(c%4), 4*(c%4)+4).
Each core computes its 4 heads' attention plus its partial out-projection
(columns of Wo for its heads); host sums the 4 partials per batch.

Layout strategy (per core):
  - Q^T, K^T computed directly in (head_dim, tokens) layout; head_dim rows are
    permuted (evens then odds) via host-permuted Wq/Wk rows so RoPE becomes a
    rotate-half over partitions 0:32/32:64 of each 64-row head block.
  - scores^T tiles (k_tokens x q) via row-packed K=64 matmuls (head pairs at
    partitions 0:64 / 64:128 of shared Q^T/K^T tiles).
  - exp on ScalarE straight out of PSUM with fused 1/sqrt(hd) scale.
  - numerator matmul uses V augmented with a ones column (M=65): row 64 of the
    PSUM accumulator is the softmax denominator.
  - normalize: DVE reciprocal of row 64, gpsimd partition-broadcast, DVE mult.
  - out-projection: lhsT = normalized attention output (already transposed),
    rhs = Wo columns for this core's heads, accumulated over 2 head-pair chunks.
"""

import numpy as np

import concourse.bass as bass
import concourse.mybir as mybir
import concourse.tile as tile
from concourse import bacc
from concourse.bass_utils import run_bass_kernel_spmd

F32 = mybir.dt.float32
F32R = mybir.dt.float32r
AF = mybir.ActivationFunctionType
OP = mybir.AluOpType

B, N, D = 2, 2048, 1024
H, HD = 16, 64
HPC = 4  # heads per core
N_CORES = 8
SCALE = HD ** -0.5

N_TOKTILES = N // 128      # 16
N_KTILES = N // 128        # 16
N_QTILES = N // 512        # 4
N_DTILES = D // 128        # 8
KG = 2                     # ktiles per exp group
NEG = -1.0e30


def _build_program(mask_all_ones: bool):
    nc = bacc.Bacc("TRN2", target_bir_lowering=False, debug=False)

    xT = nc.dram_tensor("xT", [D, N], F32R, kind="ExternalInput")
    wqT = nc.dram_tensor("wqT", [D, HPC * HD], F32R, kind="ExternalInput")
    wkT = nc.dram_tensor("wkT", [D, HPC * HD], F32R, kind="ExternalInput")
    wvT = nc.dram_tensor("wvT", [D, HPC * HD], F32R, kind="ExternalInput")
    woT = nc.dram_tensor("woT", [HPC * HD, D], F32R, kind="ExternalInput")
    cosT = nc.dram_tensor("cosT", [128, N], F32, kind="ExternalInput")
    sinT = nc.dram_tensor("sinT", [128, N], F32, kind="ExternalInput")
    mmul = nc.dram_tensor("mmul", [128, N_KTILES], F32, kind="ExternalInput")
    zpad = nc.dram_tensor("zpad", [64, N], F32R, kind="ExternalInput")
    y = nc.dram_tensor("y", [N, D], F32, kind="ExternalOutput")

    with tile.TileContext(nc) as tc:
        # ---------------- persistent pools ----------------
        with (
            tc.tile_pool(name="qk", bufs=2 * HPC) as qk_pool,
            tc.tile_pool(name="vaug", bufs=N_TOKTILES) as v_pool,
            tc.tile_pool(name="outT", bufs=2) as outT_pool,
            tc.tile_pool(name="wo", bufs=2) as wo_pool,
            tc.tile_pool(name="mm", bufs=1) as mm_pool,
            tc.tile_pool(name="tab", bufs=2) as tab_pool,
        ):
            # QTp[h], KTp[h]: (128, N) f32r; rows 0:64 = head h, 64:128 = zeros
            # (zero-padded so score matmuls have K=128 and count as HAM-busy)
            QTp = [qk_pool.tile([128, N], F32R, tag="qk", name=f"QTp{_}") for _ in range(HPC)]
            KTp = [qk_pool.tile([128, N], F32R, tag="qk", name=f"KTp{_}") for _ in range(HPC)]
            vaug = [
                v_pool.tile([128, HPC * (HD + 1)], F32R, tag="vaug", name=f"vaug{_}")
                for _ in range(N_TOKTILES)
            ]
            outT = [outT_pool.tile([128, N], F32R, tag="outT", name=f"outT{_}") for _ in range(2)]
            woT_sb = [wo_pool.tile([128, D], F32R, tag="wo", name=f"woTsb{_}") for _ in range(2)]
            mmul_sb = mm_pool.tile([128, N_KTILES], F32)
            cos_sb = tab_pool.tile([128, N], F32, tag="tab")
            sin_sb = tab_pool.tile([128, N], F32, tag="tab")

            # RoPE tables on gpsimd (needed by quarter-0 RoPE)
            nc.gpsimd.dma_start(cos_sb[:], cosT.ap()[:])
            nc.gpsimd.dma_start(sin_sb[:], sinT.ap()[:])
            if not mask_all_ones:
                nc.gpsimd.dma_start(mmul_sb[:], mmul.ap()[:])

            # ---------------- phase A: projections + RoPE (per quarter) ------
            with (
                tc.tile_pool(name="raw", bufs=6) as raw_pool,
                tc.tile_pool(name="rot", bufs=4) as rot_pool,
                tc.tile_pool(name="xt", bufs=2 * N_DTILES) as xt_pool,
                tc.tile_pool(name="w", bufs=N_DTILES) as w_pool,
                tc.tile_pool(name="on", bufs=1) as on_pool,
                tc.tile_pool(name="ppsum", bufs=4, space="PSUM") as ppsum,
                tc.tile_pool(name="vpsum", bufs=2, space="PSUM") as vpsum,
            ):
                wq_sb = [w_pool.tile([128, HPC * HD], F32R, tag="wq", name=f"wq{_}") for _ in range(N_DTILES)]
                wk_sb = [w_pool.tile([128, HPC * HD], F32R, tag="wk", name=f"wk{_}") for _ in range(N_DTILES)]
                wv_sb = [w_pool.tile([128, HPC * HD], F32R, tag="wv", name=f"wv{_}") for _ in range(N_DTILES)]
                for d in range(N_DTILES):
                    nc.sync.dma_start(wq_sb[d][:], wqT.ap()[d * 128:(d + 1) * 128, :])
                NQ = N // 4  # 512-token quarters for xT residency
                xt0 = [xt_pool.tile([128, NQ], F32R, tag="xt", name=f"xt0_{_}") for _ in range(N_DTILES)]
                for d in range(N_DTILES):
                    nc.sync.dma_start(xt0[d][:], xT.ap()[d * 128:(d + 1) * 128, 0:NQ])
                for d in range(N_DTILES):
                    nc.sync.dma_start(wk_sb[d][:], wkT.ap()[d * 128:(d + 1) * 128, :])
                for d in range(N_DTILES):
                    nc.sync.dma_start(wv_sb[d][:], wvT.ap()[d * 128:(d + 1) * 128, :])

                ones_sc = on_pool.tile([128, HPC], F32, name="ones_sc")
                nc.vector.memset(ones_sc[:], 1.0)

                for quarter in range(4):
                    qsl = slice(quarter * NQ, (quarter + 1) * NQ)
                    if quarter == 0:
                        xt = xt0
                    else:
                        xt = [xt_pool.tile([128, NQ], F32R, tag="xt", name=f"xt{_}") for _ in range(N_DTILES)]
                        for d in range(N_DTILES):
                            nc.sync.dma_start(xt[d][:], xT.ap()[d * 128:(d + 1) * 128, qsl])
                    # Q^T / K^T projections, then RoPE per (pair, quarter)
                    for ti, w_sb in ((0, wq_sb), (2, wk_sb)):
                        for ch in range(2):
                            ps = ppsum.tile([128, 512], F32, tag="ppsum", name="ps_proj")
                            for d in range(N_DTILES):
                                nc.tensor.matmul(
                                    ps[:],
                                    w_sb[d][:, ch * 128:(ch + 1) * 128],
                                    xt[d][:],
                                    start=(d == 0),
                                    stop=(d == N_DTILES - 1),
                                )
                            rq = raw_pool.tile([128, NQ], F32R, tag="raw", name="rq")
                            nc.scalar.copy(rq[:], ps[:])
                            rot = rot_pool.tile([128, NQ], F32R, tag="rot", name="rot_t")
                            for blk in range(2):
                                b0 = blk * 64
                                nc.gpsimd.dma_start(rot[b0:b0 + 32, :], rq[b0 + 32:b0 + 64, :])
                                nc.gpsimd.dma_start(rot[b0 + 32:b0 + 64, :], rq[b0:b0 + 32, :])
                            nc.vector.tensor_tensor(rq[:], rq[:], cos_sb[:, qsl], OP.mult)
                            nc.vector.tensor_tensor(rot[:], rot[:], sin_sb[:, qsl], OP.mult)
                            nc.vector.tensor_tensor(rq[:], rq[:], rot[:], OP.add)
                            dsts = (QTp, KTp)[ti // 2]
                            h0 = ch * 2
                            nc.gpsimd.dma_start(dsts[h0][0:64, qsl], rq[0:64, :])
                            nc.gpsimd.dma_start(dsts[h0 + 1][0:64, qsl], rq[64:128, :])
                    # V: out rows = tokens, cols = 4 heads x 64, strided into vaug
                    for tt in range(4):
                        g = quarter * 4 + tt
                        psv = vpsum.tile([128, HPC * HD], F32, tag="vpsum", name="ps_v")
                        for d in range(N_DTILES):
                            nc.tensor.matmul(
                                psv[:],
                                xt[d][:, tt * 128:(tt + 1) * 128],
                                wv_sb[d][:],
                                start=(d == 0),
                                stop=(d == N_DTILES - 1),
                            )
                        for h in range(HPC):
                            nc.scalar.copy(
                                vaug[g][:, h * 65:h * 65 + 64],
                                psv[:, h * 64:(h + 1) * 64],
                            )
                        nc.vector.tensor_copy(vaug[g][:, 64::65], ones_sc[:])

            # ---------------- phase B: attention + inline out-projection -----
            with (
                tc.tile_pool(name="exp", bufs=4) as exp_pool,
                tc.tile_pool(name="div", bufs=4) as div_pool,
                tc.tile_pool(name="yout", bufs=3) as y_pool,
                tc.tile_pool(name="psumS", bufs=2, space="PSUM") as psumS,
                tc.tile_pool(name="psumN", bufs=2, space="PSUM") as psumN,
                tc.tile_pool(name="psumW", bufs=1, space="PSUM") as psumW,
            ):
                # zero-pad + Wo loads on sync: queued after all phase-A loads,
                # ahead of the tmpO writes; needed from the first score matmul
                for h in range(HPC):
                    nc.sync.dma_start(QTp[h][64:128, :], zpad.ap()[:])
                    nc.sync.dma_start(KTp[h][64:128, :], zpad.ap()[:])
                for ch in range(2):
                    nc.sync.dma_start(woT_sb[ch][:], woT.ap()[ch * 128:(ch + 1) * 128, :])

                def emit_wo(q):
                    # out-projection for qtile q (deferred; runs amid later work)
                    for tt in range(4):
                        t0 = q * 512 + tt * 128
                        pw = psumW.tile([128, D], F32, tag="pw", name="pw_t")
                        for nh in range(2):
                            ns = slice(nh * 512, (nh + 1) * 512)
                            for ch in range(2):
                                nc.tensor.matmul(
                                    pw[:, ns],
                                    outT[ch][:, t0:t0 + 128],
                                    woT_sb[ch][:, ns],
                                    start=(ch == 0), stop=(ch == 1),
                                )
                        yt = y_pool.tile([128, D], F32, tag="y", name="y_t")
                        nc.vector.tensor_copy(yt[:], pw[:])
                        nc.gpsimd.dma_start(y.ap()[t0:t0 + 128, :], yt[:])

                for q in range(N_QTILES):
                    qs = slice(q * 512, (q + 1) * 512)
                    for hp in range(2):
                        if hp == 1 and q > 0:
                            emit_wo(q - 1)
                        pnE = psumN.tile([65, 512], F32, tag="pn", name="pnE")
                        pnO = psumN.tile([65, 512], F32, tag="pn", name="pnO")
                        hE, hO = 2 * hp, 2 * hp + 1
                        for kg in range(N_KTILES // KG):
                            psE = psumS.tile([128, 512 * KG], F32, tag="ps", name="psE")
                            psO = psumS.tile([128, 512 * KG], F32, tag="ps", name="psO")
                            for j in range(KG):
                                kt = kg * KG + j
                                ks = slice(kt * 128, (kt + 1) * 128)
                                js = slice(j * 512, (j + 1) * 512)
                                # scores^T, K padded to 128 (rows 64:128 zero)
                                nc.tensor.matmul(
                                    psE[:, js], KTp[hE][:, ks], QTp[hE][:, qs],
                                    start=True, stop=True,
                                )
                                nc.tensor.matmul(
                                    psO[:, js], KTp[hO][:, ks], QTp[hO][:, qs],
                                    start=True, stop=True,
                                )
                            eE = exp_pool.tile([128, 512 * KG], F32R, tag="exp", name="eE")
                            eO = exp_pool.tile([128, 512 * KG], F32R, tag="exp", name="eO")
                            nc.scalar.activation(eE[:], psE[:], AF.Exp, scale=SCALE)
                            nc.scalar.activation(eO[:], psO[:], AF.Exp, scale=SCALE)
                            if not mask_all_ones:
                                for j in range(KG):
                                    kt = kg * KG + j
                                    js = slice(j * 512, (j + 1) * 512)
                                    nc.vector.tensor_scalar_mul(
                                        eE[:, js], eE[:, js], mmul_sb[:, kt:kt + 1]
                                    )
                                    nc.vector.tensor_scalar_mul(
                                        eO[:, js], eO[:, js], mmul_sb[:, kt:kt + 1]
                                    )
                            for j in range(KG):
                                kt = kg * KG + j
                                js = slice(j * 512, (j + 1) * 512)
                                nc.tensor.matmul(
                                    pnE[:], vaug[kt][:, hE * 65:(hE + 1) * 65],
                                    eE[:, js],
                                    start=(kt == 0), stop=(kt == N_KTILES - 1),
                                )
                                nc.tensor.matmul(
                                    pnO[:], vaug[kt][:, hO * 65:(hO + 1) * 65],
                                    eO[:, js],
                                    start=(kt == 0), stop=(kt == N_KTILES - 1),
                                )
                        # stage accumulators out of PSUM first (releases the
                        # pn banks for the next iteration), then normalize
                        stgs = []
                        for pn in (pnE, pnO):
                            stg = div_pool.tile([65, 512], F32, tag="stg", bufs=4, name="stg_t")
                            nc.vector.tensor_copy(stg[:], pn[:])
                            stgs.append(stg)
                        recs = []
                        for stg in stgs:
                            rec = div_pool.tile([1, 512], F32, tag="rec", bufs=2, name="rec_t")
                            nc.vector.reciprocal(rec[:], stg[64:65, :])
                            recs.append(rec)
                        for i, (stg, rec) in enumerate(zip(stgs, recs)):
                            rbc = div_pool.tile([64, 512], F32, tag="rbc", bufs=2, name="rbc_t")
                            nc.gpsimd.partition_broadcast(rbc[:], rec[:])
                            if i == 0:
                                nc.vector.tensor_tensor(
                                    outT[hp][0:64, qs], stg[0:64, :], rbc[:], OP.mult
                                )
                            else:
                                tmp = div_pool.tile([64, 512], F32R, tag="tmp", bufs=2, name="tmp_t")
                                nc.vector.tensor_tensor(
                                    tmp[:], stg[0:64, :], rbc[:], OP.mult
                                )
                                nc.sync.dma_start(outT[hp][64:128, qs], tmp[:])
                emit_wo(N_QTILES - 1)

    nc.compile()
    return nc


_CACHE = {}


def _get_program(mask_all_ones: bool):
    if mask_all_ones not in _CACHE:
        _CACHE[mask_all_ones] = _build_program(mask_all_ones)
    return _CACHE[mask_all_ones]


def _host_inputs(x, mask, Wq, Wk, Wv, Wo):
    """Build the 8 per-core input maps."""
    x = np.asarray(x, np.float32)
    mask = np.asarray(mask)
    Wq, Wk, Wv, Wo = (np.asarray(w, np.float32) for w in (Wq, Wk, Wv, Wo))

    # RoPE tables in rotate-half permuted space, repeated per 64-row block
    inv_freq = 1.0 / (10000.0 ** (np.arange(0, HD, 2, dtype=np.float32) / HD))
    ang = np.outer(np.arange(N, dtype=np.float32), inv_freq)  # (N, 32)
    cos = np.cos(ang).T.astype(np.float32)  # (32, N)
    sin = np.sin(ang).T.astype(np.float32)
    cosT = np.concatenate([cos, cos, cos, cos], 0)  # (128, N)
    sinT = np.concatenate([-sin, sin, -sin, sin], 0)

    perm = np.concatenate([np.arange(0, HD, 2), np.arange(1, HD, 2)])  # evens|odds

    xTs = [np.ascontiguousarray(x[b].T) for b in range(B)]
    in_maps = []
    for c in range(N_CORES):
        b, g = divmod(c, HPC)
        rows = []
        for h in range(HPC):
            h_abs = g * HPC + h
            rows.append(h_abs * HD + perm)
        rows = np.concatenate(rows)  # 256 permuted row indices
        vrows = np.arange(g * HPC * HD, (g + 1) * HPC * HD)  # unpermuted
        mb = mask[b].astype(np.float32).reshape(N_KTILES, 128).T.copy()  # (128,16)
        in_maps.append({
            "xT": xTs[b],
            "wqT": np.ascontiguousarray(Wq[rows].T),
            "wkT": np.ascontiguousarray(Wk[rows].T),
            "wvT": np.ascontiguousarray(Wv[vrows].T),
            "woT": np.ascontiguousarray(Wo[:, vrows].T),
            "cosT": cosT,
            "sinT": sinT,
            "mmul": np.ascontiguousarray(mb),
            "zpad": np.zeros((64, N), np.float32),
        })
    return in_maps


def kernel(x, mask, Wq, Wk, Wv, Wo, _want_profile=False):
    mask_all_ones = bool(np.asarray(mask).all())
    nc = _get_program(mask_all_ones)
    in_maps = _host_inputs(x, mask, Wq, Wk, Wv, Wo)
    kw = {}
    if _want_profile:
        import os
        import shutil
        shutil.rmtree("/root/problem/prof", ignore_errors=True)
        os.makedirs("/root/problem/prof", exist_ok=True)
        kw["tmpdir"] = "/root/problem/prof"
    res = run_bass_kernel_spmd(
        nc, in_maps, list(range(N_CORES)), trace=_want_profile, **kw
    )
    out = np.zeros((B, N, D), np.float32)
    for c in range(N_CORES):
        out[c // HPC] += res.results[c]["y"]
    if _want_profile:
        return out, res
    return out
